# Initial kernel scaffold
#
"""Trainium2 Bass kernel for nn_Equi_Nonlin_Grad_Module (fwd + input-grad).

Contract: kernel(**inputs) takes the FULL inputs from setup_inputs() and
returns the FULL output (x [2048,256], y [2048,480]) as the reference does.

Strategy (data-parallel over rows, 8 cores x 256 rows):
  forward FCTP   : hpre[b,h] = sum_uv OPT[uv,b] * w'[uv,h]   (PSUM-accumulated
                   GEMM over 168 k-tiles; OPT outer-product tiles built on-chip)
  MLP fwd/bwd    : PE GEMMs + LN via fused stat ops + SiLU via Sigmoid
  backward FCTP  : S[b,uv] = g_h[b,:] @ wsymT'[:,uv]  (GEMM, streamed fp16
                   pre-transposed symmetrized weights) then per-row
                   mul+reduce against x on DVE.
Weights are cast/symmetrized/pre-scaled to fp16 on the host (marshalling);
all matmuls accumulate in fp32 PSUM.
"""
import math
from contextlib import ExitStack

import numpy as np

N, HID, Z, DIM = 2048, 1024, 256, 480
NCORES = 8
B = N // NCORES          # rows per core (256)
NBT = B // 128           # b-tiles per core (2)
FAN = 128 * 128 + 64 * 64 + 32 * 32
EPS = 1e-6

_cache = {}


def _build(nc, tile, mybir, masks, with_exitstack):
    F32 = mybir.dt.float32
    F16 = mybir.dt.float16
    AF = mybir.ActivationFunctionType
    OP = mybir.AluOpType
    AX = mybir.AxisListType
    P = 128

    # ---------------- DRAM tensors ----------------
    def din(name, shape, dt=F16):
        return nc.dram_tensor(name, shape, dt, kind="ExternalInput").ap()

    t_in = din("t_in", [B, DIM], F32)
    w0r = din("w0r", [16384, HID]); w1r = din("w1r", [4096, HID]); w2r = din("w2r", [1024, HID])
    w0s = din("w0s", [HID, 16384]); w1s = din("w1s", [HID, 4096]); w2s = din("w2s", [HID, 1024])
    W1d = din("W1d", [HID, HID]); W1Td = din("W1Td", [HID, HID])
    W2d = din("W2d", [HID, Z]); W2Td = din("W2Td", [Z, HID])
    b1d = din("b1d", [1, HID], F32); g1d = din("g1d", [1, HID], F32); be1d = din("be1d", [1, HID], F32)
    b2d = din("b2d", [1, Z], F32); g2d = din("g2d", [1, Z], F32); be2d = din("be2d", [1, Z], F32)
    xout_d = nc.dram_tensor("xout_d", [B, Z], F32, kind="ExternalOutput").ap()
    y_d = nc.dram_tensor("y_d", [B, DIM], F32, kind="ExternalOutput").ap()

    @with_exitstack
    def body(ctx: ExitStack, tc):
        sb = ctx.enter_context(tc.tile_pool(name="sb", bufs=1))

        # ---------------- one-time setup ----------------
        ident = sb.tile([P, P], F16)
        masks.make_identity(nc, ident[:])

        # broadcast bias/gain rows to all partitions (fp16 for gains, fp32 bias)
        def bcast_row(dram, width, name):
            row32 = sb.tile([1, width], F32, name=f"{name}_r32")
            nc.sync.dma_start(row32[:], dram[:])
            out = sb.tile([P, width], F32, name=f"{name}_bc")
            nc.gpsimd.partition_broadcast(out[:], row32[:])
            return out

        b1bc = bcast_row(b1d, HID, "b1")
        g1bc = bcast_row(g1d, HID, "g1")
        be1bc = bcast_row(be1d, HID, "be1")
        b2bc = bcast_row(b2d, Z, "b2")
        g2bc = bcast_row(g2d, Z, "g2")
        be2bc = bcast_row(be2d, Z, "be2")

        # ---------------- load t, build x views ----------------
        t_sb = [sb.tile([P, DIM], F32, name=f"t_sb{bt}") for bt in range(NBT)]
        for bt in range(NBT):
            nc.sync.dma_start(t_sb[bt][:], t_in[bt * P:(bt + 1) * P, :])
        # fp16 copies, m-grouped for l1/l2
        x0_16 = [sb.tile([P, 128], F16, name=f"x0_16_{bt}") for bt in range(NBT)]
        x1g16 = [sb.tile([P, 192], F16, name=f"x1g16_{bt}") for bt in range(NBT)]
        x2g16 = [sb.tile([P, 160], F16, name=f"x2g16_{bt}") for bt in range(NBT)]
        for bt in range(NBT):
            nc.vector.tensor_copy(x0_16[bt][:], t_sb[bt][:, 0:128])
            nc.vector.tensor_copy(
                x1g16[bt][:],
                t_sb[bt][:, 128:320].rearrange("p (u m) -> p (m u)", m=3))
            nc.vector.tensor_copy(
                x2g16[bt][:],
                t_sb[bt][:, 320:480].rearrange("p (u m) -> p (m u)", m=5))

        tp_ps = ctx.enter_context(tc.tile_pool(name="tp_ps", bufs=2, space="PSUM"))

        def transpose128(dst_ap, src_ap, fdim):
            """src [128, fdim<=128] fp16 -> dst[..] = src.T via PE."""
            ps = tp_ps.tile([fdim, P], F16, name="tp_ps_t")
            nc.tensor.transpose(ps[:], src_ap, ident[:, 0:fdim])
            nc.scalar.copy(dst_ap, ps[:])

        # transposed x tiles
        x0T = sb.tile([P, B], F16)
        x1s = [sb.tile([P, B], F16, name=f"x1s{m}") for m in range(3)]
        x2s = [sb.tile([P, B], F16, name=f"x2s{m}") for m in range(5)]
        for bt in range(NBT):
            transpose128(x0T[:, bt * P:(bt + 1) * P], x0_16[bt][:], 128)
            for m in range(3):
                transpose128(x1s[m][0:64, bt * P:(bt + 1) * P],
                             x1g16[bt][:, 64 * m:64 * (m + 1)], 64)
            for m in range(5):
                transpose128(x2s[m][0:32, bt * P:(bt + 1) * P],
                             x2g16[bt][:, 32 * m:32 * (m + 1)], 32)
        # duplicate partitions for the stacked tiles
        for m in range(3):
            nc.sync.dma_start(x1s[m][64:128, :], x1s[m][0:64, :])
        for m in range(5):
            for r in range(1, 4):
                nc.sync.dma_start(x2s[m][32 * r:32 * (r + 1), :], x2s[m][0:32, :])

        # ---------------- forward FCTP GEMM ----------------
        # hpre[b,h] accumulated in PSUM over 168 k-tiles
        fw_ps = ctx.enter_context(tc.tile_pool(name="fw_ps", bufs=1, space="PSUM"))
        hpre_ps = [fw_ps.tile([P, HID], F32, name=f"hpre_ps{bt}") for bt in range(NBT)]

        wpool = ctx.enter_context(tc.tile_pool(name="wpool", bufs=4))
        bps_pool = ctx.enter_context(tc.tile_pool(name="bps", bufs=2, space="PSUM"))
        b16_pool = ctx.enter_context(tc.tile_pool(name="b16", bufs=4))
        opt_pool = ctx.enter_context(tc.tile_pool(name="opt", bufs=8))
        KT = 128 + 32 + 8

        def emit_fwd_mms(opt_t, w_t, kk):
            for bt in range(NBT):
                for hb in range(2):
                    nc.tensor.matmul(
                        hpre_ps[bt][:, hb * 512:(hb + 1) * 512],
                        opt_t[:, bt * P:(bt + 1) * P],
                        w_t[:, hb * 512:(hb + 1) * 512],
                        start=(kk == 0), stop=(kk == KT - 1))

        kk = 0
        # l0: k-tile per u
        for u in range(128):
            w_t = wpool.tile([P, HID], F16, name="w_t")
            nc.sync.dma_start(w_t[:], w0r[u * P:(u + 1) * P, :])
            bps = bps_pool.tile([P, B], F32, name="bps")
            nc.tensor.matmul(bps[:], ident[:, u:u + 1].broadcast_to([P, P]), x0T[:])
            b16 = b16_pool.tile([P, B], F16, name="b16")
            nc.scalar.copy(b16[:], bps[:])
            opt_t = opt_pool.tile([P, B], F16, name="opt_t")
            nc.vector.tensor_mul(opt_t[:], x0T[:], b16[:])
            emit_fwd_mms(opt_t, w_t, kk); kk += 1
        # l1: k-tile per u-pair
        for j in range(32):
            w_t = wpool.tile([P, HID], F16, name="w_t")
            nc.sync.dma_start(w_t[:], w1r[j * P:(j + 1) * P, :])
            opt_t = opt_pool.tile([P, B], F16, name="opt_t")
            for m in range(3):
                bps = bps_pool.tile([P, B], F32, name="bps")
                nc.tensor.matmul(
                    bps[:],
                    ident[0:64, 2 * j:2 * j + 2].unsqueeze(2).broadcast_to([64, 2, 64]),
                    x1s[m][0:64, :])
                b16 = b16_pool.tile([P, B], F16, name="b16")
                nc.scalar.copy(b16[:], bps[:])
                if m == 0:
                    nc.vector.tensor_mul(opt_t[:], x1s[m][:], b16[:])
                else:
                    tmp = b16_pool.tile([P, B], F16, name="b16")
                    nc.vector.tensor_mul(tmp[:], x1s[m][:], b16[:])
                    nc.gpsimd.tensor_add(opt_t[:], opt_t[:], tmp[:])
            emit_fwd_mms(opt_t, w_t, kk); kk += 1
        # l2: k-tile per u-quad
        for j in range(8):
            w_t = wpool.tile([P, HID], F16, name="w_t")
            nc.sync.dma_start(w_t[:], w2r[j * P:(j + 1) * P, :])
            opt_t = opt_pool.tile([P, B], F16, name="opt_t")
            for m in range(5):
                bps = bps_pool.tile([P, B], F32, name="bps")
                nc.tensor.matmul(
                    bps[:],
                    ident[0:32, 4 * j:4 * j + 4].unsqueeze(2).broadcast_to([32, 4, 32]),
                    x2s[m][0:32, :])
                b16 = b16_pool.tile([P, B], F16, name="b16")
                nc.scalar.copy(b16[:], bps[:])
                if m == 0:
                    nc.vector.tensor_mul(opt_t[:], x2s[m][:], b16[:])
                else:
                    tmp = b16_pool.tile([P, B], F16, name="b16")
                    nc.vector.tensor_mul(tmp[:], x2s[m][:], b16[:])
                    nc.gpsimd.tensor_add(opt_t[:], opt_t[:], tmp[:])
            emit_fwd_mms(opt_t, w_t, kk); kk += 1
        assert kk == KT

        # hpre psum -> fp16 + transposed k-tiles for GEMM2
        hpre16 = [sb.tile([P, HID], F16, name=f"hpre16_{bt}") for bt in range(NBT)]
        for bt in range(NBT):
            nc.vector.tensor_copy(hpre16[bt][:], hpre_ps[bt][:])
        hpreT = [sb.tile([P, B], F16, name=f"hpreT{k}") for k in range(8)]
        for bt in range(NBT):
            for k in range(8):
                transpose128(hpreT[k][:, bt * P:(bt + 1) * P],
                             hpre16[bt][:, k * P:(k + 1) * P], 128)

        # ---------------- MLP weights resident ----------------
        W1sb = [sb.tile([P, HID], F16, name=f"W1sb{k}") for k in range(8)]
        W2sb = [sb.tile([P, Z], F16, name=f"W2sb{k}") for k in range(8)]
        W1Tsb = [sb.tile([P, HID], F16, name=f"W1Tsb{k}") for k in range(8)]
        W2Tsb = [sb.tile([P, HID], F16, name=f"W2Tsb{k}") for k in range(2)]
        for k in range(8):
            nc.sync.dma_start(W1sb[k][:], W1d[k * P:(k + 1) * P, :])
            nc.sync.dma_start(W2sb[k][:], W2d[k * P:(k + 1) * P, :])
            nc.sync.dma_start(W1Tsb[k][:], W1Td[k * P:(k + 1) * P, :])
        for k in range(2):
            nc.sync.dma_start(W2Tsb[k][:], W2Td[k * P:(k + 1) * P, :])

        mlp_ps = ctx.enter_context(tc.tile_pool(name="mlp_ps", bufs=2, space="PSUM"))
        scr32 = ctx.enter_context(tc.tile_pool(name="scr32", bufs=4))
        tiny = ctx.enter_context(tc.tile_pool(name="tiny", bufs=16))

        eps_ap = sb.tile([P, 1], F32)
        nc.vector.memset(eps_ap[:], EPS)

        def layer_norm(a_ps, bias_bc, gain_bc, beta_bc, width, tag):
            """psum a -> (xh fp32, rs [P,1], l fp32) ; a = psum + bias."""
            a_t = scr32.tile([P, width], F32, name=f"a_{tag}", tag=f"a_{tag}")
            sumx = tiny.tile([P, 1], F32, name=f"sumx_{tag}")
            nc.vector.scalar_tensor_tensor(
                out=a_t[:], in0=a_ps[:], scalar=1.0, in1=bias_bc[:],
                op0=OP.mult, op1=OP.add, accum_out=sumx[:])
            sq = scr32.tile([P, width], F16, name=f"sq_{tag}", tag=f"sq_{tag}")
            sumsq = tiny.tile([P, 1], F32, name=f"sumsq_{tag}")
            nc.scalar.activation(sq[:], a_t[:], AF.Square, accum_out=sumsq[:])
            mu = tiny.tile([P, 1], F32, name=f"mu_{tag}")
            nc.vector.tensor_scalar_mul(mu[:], sumx[:], 1.0 / width)
            var = tiny.tile([P, 1], F32, name=f"var_{tag}")
            nc.vector.tensor_scalar_mul(var[:], sumsq[:], 1.0 / width)
            musq = tiny.tile([P, 1], F32, name=f"musq_{tag}")
            nc.vector.tensor_mul(musq[:], mu[:], mu[:])
            nc.vector.tensor_sub(var[:], var[:], musq[:])
            sd = tiny.tile([P, 1], F32, name=f"sd_{tag}")
            nc.scalar.activation(sd[:], var[:], AF.Sqrt, bias=eps_ap[:])
            rs = tiny.tile([P, 1], F32, name=f"rs_{tag}")
            nc.vector.reciprocal(rs[:], sd[:])
            nmrs = tiny.tile([P, 1], F32, name=f"nmrs_{tag}")
            nc.vector.tensor_mul(nmrs[:], mu[:], rs[:])
            nc.vector.tensor_scalar_mul(nmrs[:], nmrs[:], -1.0)
            xh = scr32.tile([P, width], F32, name=f"xh_{tag}", tag=f"xh_{tag}")
            nc.scalar.activation(xh[:], a_t[:], AF.Identity, bias=nmrs[:], scale=rs[:])
            l_t = scr32.tile([P, width], F32, name=f"l_{tag}", tag=f"l_{tag}")
            nc.vector.tensor_mul(l_t[:], xh[:], gain_bc[:])
            nc.vector.tensor_add(l_t[:], l_t[:], beta_bc[:])
            return a_t, xh, rs, l_t

        # ---- layer 1 fwd ----
        xh1 = [None] * NBT; rs1 = [None] * NBT; sig1 = [None] * NBT; s116 = [None] * NBT
        for bt in range(NBT):
            a1_ps = mlp_ps.tile([P, HID], F32, name="a1_ps", tag="mlp_ps_big")
            for k in range(8):
                for hb in range(2):
                    nc.tensor.matmul(
                        a1_ps[:, hb * 512:(hb + 1) * 512],
                        hpreT[k][:, bt * P:(bt + 1) * P],
                        W1sb[k][:, hb * 512:(hb + 1) * 512],
                        start=(k == 0), stop=(k == 7))
            _, xh, rs, l_t = layer_norm(a1_ps, b1bc, g1bc, be1bc, HID, f"l1b{bt}")
            sg = scr32.tile([P, HID], F32, name=f"sig1_{bt}", tag=f"sig1_{bt}")
            nc.scalar.activation(sg[:], l_t[:], AF.Sigmoid)
            s_t = sb.tile([P, HID], F16, name=f"s116_{bt}")
            nc.vector.tensor_mul(s_t[:], l_t[:], sg[:])
            xh1[bt], rs1[bt], sig1[bt], s116[bt] = xh, rs, sg, s_t

        s1T = [sb.tile([P, B], F16, name=f"s1T{k}") for k in range(8)]
        for bt in range(NBT):
            for k in range(8):
                transpose128(s1T[k][:, bt * P:(bt + 1) * P],
                             s116[bt][:, k * P:(k + 1) * P], 128)

        # ---- layer 2 fwd + bwd head ----
        da2T = [sb.tile([P, B], F16, name=f"da2T{k}") for k in range(2)]
        for bt in range(NBT):
            a2_ps = mlp_ps.tile([P, Z], F32, name="a2_ps", tag="mlp_ps_small")
            for k in range(8):
                nc.tensor.matmul(a2_ps[:], s1T[k][:, bt * P:(bt + 1) * P],
                                 W2sb[k][:], start=(k == 0), stop=(k == 7))
            _, xh2, rs2, l2 = layer_norm(a2_ps, b2bc, g2bc, be2bc, Z, f"l2b{bt}")
            sig2 = scr32.tile([P, Z], F32, name="sig2", tag="sig2")
            nc.scalar.activation(sig2[:], l2[:], AF.Sigmoid)
            xo = scr32.tile([P, Z], F32, name="xo", tag="xo")
            nc.vector.tensor_mul(xo[:], l2[:], sig2[:])
            nc.sync.dma_start(xout_d[bt * P:(bt + 1) * P, :], xo[:])
            # dl2 = silu'(l2) = sig2 + xo - xo*sig2
            t1 = scr32.tile([P, Z], F32, name="t1", tag="dl2t1")
            nc.vector.tensor_mul(t1[:], xo[:], sig2[:])
            dl2 = scr32.tile([P, Z], F32, name="dl2", tag="dl2")
            nc.vector.tensor_add(dl2[:], sig2[:], xo[:])
            nc.vector.tensor_sub(dl2[:], dl2[:], t1[:])
            # LN2 backward
            p2 = scr32.tile([P, Z], F32, name="p2", tag="p2")
            m1s = tiny.tile([P, 1], F32, name="m1s2")
            nc.vector.tensor_tensor_reduce(
                out=p2[:], in0=dl2[:], in1=g2bc[:], scale=1.0, scalar=0.0,
                op0=OP.mult, op1=OP.add, accum_out=m1s[:])
            pxh = scr32.tile([P, Z], F16, name="pxh2", tag="pxh2")
            m2s = tiny.tile([P, 1], F32, name="m2s2")
            nc.vector.tensor_tensor_reduce(
                out=pxh[:], in0=p2[:], in1=xh2[:], scale=1.0, scalar=0.0,
                op0=OP.mult, op1=OP.add, accum_out=m2s[:])
            m1 = tiny.tile([P, 1], F32, name="m1_2")
            nc.vector.tensor_scalar_mul(m1[:], m1s[:], 1.0 / Z)
            m2 = tiny.tile([P, 1], F32, name="m2_2")
            nc.vector.tensor_scalar_mul(m2[:], m2s[:], 1.0 / Z)
            dpre = scr32.tile([P, Z], F32, name="dpre2", tag="dpre2")
            nc.vector.ln_bwd_dx(out=dpre[:], dy=p2[:], x_hat=xh2[:],
                                mean_dyx=m2[:], mean_dy=m1[:])
            da216 = scr32.tile([P, Z], F16, name="da216", tag="da216")
            nc.scalar.activation(da216[:], dpre[:], AF.Copy, scale=rs2[:])
            for k in range(2):
                transpose128(da2T[k][:, bt * P:(bt + 1) * P],
                             da216[:, k * P:(k + 1) * P], 128)

        # ---- backward through layer 1 ----
        da1T = [sb.tile([P, B], F16, name=f"da1T{k}") for k in range(8)]
        for bt in range(NBT):
            ds1_ps = mlp_ps.tile([P, HID], F32, name="ds1_ps", tag="mlp_ps_big")
            for hb in range(2):
                for k in range(2):
                    nc.tensor.matmul(
                        ds1_ps[:, hb * 512:(hb + 1) * 512],
                        da2T[k][:, bt * P:(bt + 1) * P],
                        W2Tsb[k][:, hb * 512:(hb + 1) * 512],
                        start=(k == 0), stop=(k == 1))
            # silu'(l1) = sig1 + s1 - s1*sig1
            t1 = scr32.tile([P, HID], F32, name="t1b", tag="dslt1")
            nc.vector.tensor_mul(t1[:], s116[bt][:], sig1[bt][:])
            dsl = scr32.tile([P, HID], F32, name="dsl", tag="dsl")
            nc.vector.tensor_add(dsl[:], sig1[bt][:], s116[bt][:])
            nc.vector.tensor_sub(dsl[:], dsl[:], t1[:])
            dl1 = scr32.tile([P, HID], F32, name="dl1", tag="dl1")
            nc.vector.tensor_mul(dl1[:], ds1_ps[:], dsl[:])
            p1 = scr32.tile([P, HID], F32, name="p1", tag="p1")
            m1s = tiny.tile([P, 1], F32, name="m1s1")
            nc.vector.tensor_tensor_reduce(
                out=p1[:], in0=dl1[:], in1=g1bc[:], scale=1.0, scalar=0.0,
                op0=OP.mult, op1=OP.add, accum_out=m1s[:])
            pxh = scr32.tile([P, HID], F16, name="pxh1", tag="pxh1")
            m2s = tiny.tile([P, 1], F32, name="m2s1")
            nc.vector.tensor_tensor_reduce(
                out=pxh[:], in0=p1[:], in1=xh1[bt][:], scale=1.0, scalar=0.0,
                op0=OP.mult, op1=OP.add, accum_out=m2s[:])
            m1 = tiny.tile([P, 1], F32, name="m1_1")
            nc.vector.tensor_scalar_mul(m1[:], m1s[:], 1.0 / HID)
            m2 = tiny.tile([P, 1], F32, name="m2_1")
            nc.vector.tensor_scalar_mul(m2[:], m2s[:], 1.0 / HID)
            dpre = scr32.tile([P, HID], F32, name="dpre1", tag="dpre1")
            nc.vector.ln_bwd_dx(out=dpre[:], dy=p1[:], x_hat=xh1[bt][:],
                                mean_dyx=m2[:], mean_dy=m1[:])
            da116 = scr32.tile([P, HID], F16, name="da116", tag="da116")
            nc.scalar.activation(da116[:], dpre[:], AF.Copy, scale=rs1[bt][:])
            for k in range(8):
                transpose128(da1T[k][:, bt * P:(bt + 1) * P],
                             da116[:, k * P:(k + 1) * P], 128)

        # g_hT[i, b] = sum_j W1T[j, i] * da1T[j, b]
        ghT = [sb.tile([P, B], F16, name=f"ghT{i}") for i in range(8)]
        for i in range(8):
            gh_ps = mlp_ps.tile([P, B], F32, name="gh_ps", tag="mlp_ps_small")
            for k in range(8):
                nc.tensor.matmul(gh_ps[:], W1Tsb[k][:, i * P:(i + 1) * P],
                                 da1T[k][:], start=(k == 0), stop=(k == 7))
            nc.scalar.copy(ghT[i][:], gh_ps[:])

        # ---------------- backward FCTP (S GEMMs + mul-reduce) ----------------
        y_sb = [sb.tile([P, DIM], F32, name=f"y_sb{bt}") for bt in range(NBT)]
        ws_pool = ctx.enter_context(tc.tile_pool(name="ws_pool", bufs=3))
        s_ps_pool = ctx.enter_context(tc.tile_pool(name="s_ps", bufs=4, space="PSUM"))
        red_pool = ctx.enter_context(tc.tile_pool(name="red", bufs=6))

        def s_gemm_block(wsd, nb, ncols):
            """DMA [1024, 512] block of wsd at col nb*512 -> S psum per bt."""
            wst = ws_pool.tile([P, 8, 512], F16, name="wst")
            nc.sync.dma_start(
                wst[:],
                wsd[:, nb * 512:(nb + 1) * 512]
                .rearrange("(k p) c -> p k c", p=P))
            s_ps = []
            for bt in range(NBT):
                ps = s_ps_pool.tile([P, 512], F32, name="s_ps_t")
                for k in range(8):
                    nc.tensor.matmul(ps[:], ghT[k][:, bt * P:(bt + 1) * P],
                                     wst[:, k, :], start=(k == 0), stop=(k == 7))
                s_ps.append(ps)
            return s_ps

        # l0: 32 blocks, block nb covers u in [4nb, 4nb+4), v full 128
        for nb in range(32):
            s_ps = s_gemm_block(w0s, nb, 512)
            for bt in range(NBT):
                s16 = red_pool.tile([P, 512], F16, name="s16", tag="s16")
                nc.scalar.copy(s16[:], s_ps[bt][:])
                tmp = red_pool.tile([P, 4, 128], F16, name="tmpr", tag="tmpr")
                nc.gpsimd.tensor_mul(
                    tmp[:], s16[:].rearrange("p (u v) -> p u v", u=4),
                    x0_16[bt][:].unsqueeze(1).broadcast_to([P, 4, 128]))
                nc.vector.reduce_sum(y_sb[bt][:, 4 * nb:4 * nb + 4], tmp[:], axis=AX.X)
        # l1: 8 blocks, block nb covers u in [8nb, 8nb+8), v 64; y cols 128 + 3u + m
        for nb in range(8):
            s_ps = s_gemm_block(w1s, nb, 512)
            for bt in range(NBT):
                s16 = red_pool.tile([P, 512], F16, name="s16", tag="s16")
                nc.scalar.copy(s16[:], s_ps[bt][:])
                for m in range(3):
                    tmp = red_pool.tile([P, 8, 64], F16, name="tmpr1", tag="tmpr")
                    nc.gpsimd.tensor_mul(
                        tmp[:], s16[:].rearrange("p (u v) -> p u v", u=8),
                        t_sb[bt][:, 128 + m:320:3].unsqueeze(1).broadcast_to([P, 8, 64]))
                    nc.vector.reduce_sum(
                        y_sb[bt][:, 128 + 24 * nb + m:128 + 24 * (nb + 1):3],
                        tmp[:], axis=AX.X)
        # l2: 2 blocks, block nb covers u in [16nb, 16nb+16), v 32; y cols 320 + 5u + m
        for nb in range(2):
            s_ps = s_gemm_block(w2s, nb, 512)
            for bt in range(NBT):
                s16 = red_pool.tile([P, 512], F16, name="s16", tag="s16")
                nc.scalar.copy(s16[:], s_ps[bt][:])
                for m in range(5):
                    tmp = red_pool.tile([P, 16, 32], F16, name="tmpr2", tag="tmpr")
                    nc.gpsimd.tensor_mul(
                        tmp[:], s16[:].rearrange("p (u v) -> p u v", u=16),
                        t_sb[bt][:, 320 + m:480:5].unsqueeze(1).broadcast_to([P, 16, 32]))
                    nc.vector.reduce_sum(
                        y_sb[bt][:, 320 + 80 * nb + m:320 + 80 * (nb + 1):5],
                        tmp[:], axis=AX.X)

        for bt in range(NBT):
            nc.sync.dma_start(y_d[bt * P:(bt + 1) * P, :], y_sb[bt][:])

    return body


def build_program():
    import concourse.bass as bass
    import concourse.tile as tile
    import concourse.mybir as mybir
    from concourse import masks
    from concourse._compat import with_exitstack

    nc = bass.Bass("TRN2", target_bir_lowering=False, debug=False,
                   num_devices=NCORES)
    body = _build(nc, tile, mybir, masks, with_exitstack)
    with tile.TileContext(nc) as tc:
        body(tc)
    return nc


def prep_inputs(tensor_in, w0, w1, w2, W1, b1, g1, be1, W2, b2, g2, be2):
    """Host-side marshalling: scale/symmetrize/cast/transpose weights."""
    sF = math.sqrt(float(FAN))
    f16 = np.float16
    w0_ = (w0 / sF)
    w1_ = (w1 / (math.sqrt(3.0) * sF))
    w2_ = (w2 / (math.sqrt(5.0) * sF))
    common = {
        "w0r": np.ascontiguousarray(w0_.reshape(16384, HID), f16),
        "w1r": np.ascontiguousarray(w1_.reshape(4096, HID), f16),
        "w2r": np.ascontiguousarray(w2_.reshape(1024, HID), f16),
        "w0s": np.ascontiguousarray(
            (w0_ + w0_.transpose(1, 0, 2)).reshape(16384, HID).T, f16),
        "w1s": np.ascontiguousarray(
            (w1_ + w1_.transpose(1, 0, 2)).reshape(4096, HID).T, f16),
        "w2s": np.ascontiguousarray(
            (w2_ + w2_.transpose(1, 0, 2)).reshape(1024, HID).T, f16),
        "W1d": np.ascontiguousarray(W1, f16),
        "W1Td": np.ascontiguousarray(W1.T, f16),
        "W2d": np.ascontiguousarray(W2, f16),
        "W2Td": np.ascontiguousarray(W2.T, f16),
        "b1d": np.ascontiguousarray(b1.reshape(1, HID), np.float32),
        "g1d": np.ascontiguousarray(g1.reshape(1, HID), np.float32),
        "be1d": np.ascontiguousarray(be1.reshape(1, HID), np.float32),
        "b2d": np.ascontiguousarray(b2.reshape(1, Z), np.float32),
        "g2d": np.ascontiguousarray(g2.reshape(1, Z), np.float32),
        "be2d": np.ascontiguousarray(be2.reshape(1, Z), np.float32),
    }
    t32 = np.ascontiguousarray(tensor_in, np.float32)
    in_maps = []
    for c in range(NCORES):
        m = dict(common)
        m["t_in"] = np.ascontiguousarray(t32[c * B:(c + 1) * B, :])
        in_maps.append(m)
    return in_maps


def kernel(**inputs):
    from concourse import bass_utils

    if "nc" not in _cache:
        _cache["nc"] = build_program()
    nc = _cache["nc"]
    in_maps = prep_inputs(**inputs)
    res = bass_utils.run_bass_kernel_spmd(nc, in_maps, core_ids=list(range(NCORES)))
    x = np.concatenate([res.results[c]["xout_d"] for c in range(NCORES)], axis=0)
    y = np.concatenate([res.results[c]["y_d"] for c in range(NCORES)], axis=0)
    return x.astype(np.float32), y.astype(np.float32)


# revision 12
# speedup vs baseline: 3.2182x; 3.2182x over previous
"""Trainium2 Bass kernel for nn_Equi_Nonlin_Grad_Module (fwd + input-grad).

kernel(**inputs) takes the FULL inputs from setup_inputs() and returns the
FULL output tuple (x [2048,256], y [2048,480]) like the reference.

Data-parallel over rows: 8 cores x 256 rows, weights replicated.
  forward FCTP : hpre[b,h] = sum_uv OPT[uv,b]*w'[uv,h], one PSUM-accumulated
                 GEMM over 168 k-tiles; OPT outer-product tiles built on-chip
                 (PE broadcast-matmuls + DVE muls).
  MLP fwd/bwd  : PE GEMMs, LN via fused-accum stats + ln_bwd_dx, SiLU via
                 Sigmoid.
  backward FCTP: S[b,uv] = g_h[b,:] @ wsymT'[:,uv] GEMM over streamed fp16
                 pre-transposed symmetrized weights, then mul+reduce vs x.
Weights are pre-scaled/symmetrized/cast to fp16 on the host; matmuls
accumulate in fp32 PSUM.
"""
import math
from contextlib import ExitStack

import numpy as np

N, HID, Z, DIM = 2048, 1024, 256, 480
NCORES = 8
B = N // NCORES          # rows per core (256)
NBT = B // 128           # b-tiles per core (2)
FAN = 128 * 128 + 64 * 64 + 32 * 32
EPS = 1e-6

_cache = {}


def _emit(nc, tc, tile, mybir, masks):
    F32 = mybir.dt.float32
    F16 = mybir.dt.float16
    AF = mybir.ActivationFunctionType
    OP = mybir.AluOpType
    AX = mybir.AxisListType
    P = 128

    def din(name, shape, dt=F16):
        return nc.dram_tensor(name, shape, dt, kind="ExternalInput").ap()

    t_in = din("t_in", [B, DIM], F32)
    w0r = din("w0r", [16384, HID]); w1r = din("w1r", [4096, HID]); w2r = din("w2r", [1024, HID])
    w0s = din("w0s", [HID, 16384]); w1s = din("w1s", [HID, 4096]); w2s = din("w2s", [HID, 1024])
    W1d = din("W1d", [HID, HID]); W1Td = din("W1Td", [HID, HID])
    W2d = din("W2d", [HID, Z]); W2Td = din("W2Td", [Z, HID])
    b1d = din("b1d", [1, HID], F32); g1d = din("g1d", [1, HID], F32); be1d = din("be1d", [1, HID], F32)
    b2d = din("b2d", [1, Z], F32); g2d = din("g2d", [1, Z], F32); be2d = din("be2d", [1, Z], F32)
    identd = din("identd", [128, 128], F16)
    xout_d = nc.dram_tensor("xout_d", [B, Z], F32, kind="ExternalOutput").ap()
    y_d = nc.dram_tensor("y_d", [B, DIM], F32, kind="ExternalOutput").ap()

    ctx = ExitStack()
    with ctx:
        sb = ctx.enter_context(tc.tile_pool(name="sb", bufs=1))
        tp_ps = ctx.enter_context(tc.tile_pool(name="tp_ps", bufs=2, space="PSUM"))
        tiny = ctx.enter_context(tc.tile_pool(name="tiny", bufs=1))
        scr = ctx.enter_context(tc.tile_pool(name="scr", bufs=1))

        ident = sb.tile([P, P], F16)
        nc.sync.dma_start(ident[:], identd[:])
        eps_ap = sb.tile([P, 1], F32)
        nc.vector.memset(eps_ap[:], EPS)

        ones116 = sb.tile([1, P], F16)
        nc.vector.memset(ones116[:], 1.0)
        ones132 = sb.tile([1, P], F32)
        nc.vector.memset(ones132[:], 1.0)

        def bcast_row(bc_ps, dram, width, name, dt):
            row32 = sb.tile([1, width], F32, name=f"{name}_r32")
            nc.sync.dma_start(row32[:], dram[:])
            if dt == F32:
                row, ones = row32, ones132
            else:
                row = sb.tile([1, width], dt, name=f"{name}_r16")
                nc.vector.tensor_copy(row[:], row32[:])
                ones = ones116
            out = sb.tile([P, width], dt, name=f"{name}_bc")
            for c0 in range(0, width, 512):
                cw = min(512, width - c0)
                ps = bc_ps.tile([P, 512], F32, name="bc_ps_t", tag="bc")
                nc.tensor.matmul(ps[:, 0:cw], ones[:], row[:, c0:c0 + cw],
                                 skip_group_check=True)
                nc.scalar.copy(out[:, c0:c0 + cw], ps[:, 0:cw])
            return out

        with tc.tile_pool(name="bc_ps", bufs=2, space="PSUM") as bc_ps:
            b1bc = bcast_row(bc_ps, b1d, HID, "b1", F32)
            g1bc = bcast_row(bc_ps, g1d, HID, "g1", F16)
            be1bc = bcast_row(bc_ps, be1d, HID, "be1", F16)
            b2bc = bcast_row(bc_ps, b2d, Z, "b2", F32)
            g2bc = bcast_row(bc_ps, g2d, Z, "g2", F16)
            be2bc = bcast_row(bc_ps, be2d, Z, "be2", F16)

        # ---------------- load t, build x views ----------------
        t_sb = [sb.tile([P, DIM], F32, name=f"t_sb{bt}") for bt in range(NBT)]
        x0_16 = [sb.tile([P, 128], F16, name=f"x0_16_{bt}") for bt in range(NBT)]
        x1g16 = [sb.tile([P, 192], F16, name=f"x1g16_{bt}") for bt in range(NBT)]
        x2g16 = [sb.tile([P, 160], F16, name=f"x2g16_{bt}") for bt in range(NBT)]
        for bt in range(NBT):
            nc.sync.dma_start(t_sb[bt][:], t_in[bt * P:(bt + 1) * P, :])
            nc.vector.tensor_copy(x0_16[bt][:], t_sb[bt][:, 0:128])
            nc.vector.tensor_copy(
                x1g16[bt][:].rearrange("p (m u) -> p m u", m=3),
                t_sb[bt][:, 128:320].rearrange("p (u m) -> p m u", m=3))
            nc.vector.tensor_copy(
                x2g16[bt][:].rearrange("p (m u) -> p m u", m=5),
                t_sb[bt][:, 320:480].rearrange("p (u m) -> p m u", m=5))

        def transpose128(dst_ap, src_ap, fdim):
            ps = tp_ps.tile([fdim, P], F16, name="tp_ps_t", tag="tp")
            nc.tensor.transpose(ps[:], src_ap, ident[:])
            nc.scalar.copy(dst_ap, ps[:])

        x0T = sb.tile([P, B], F16)
        x1s = [sb.tile([P, B], F16, name=f"x1s{m}") for m in range(3)]
        x2s = [sb.tile([P, B], F16, name=f"x2s{m}") for m in range(5)]
        for bt in range(NBT):
            transpose128(x0T[:, bt * P:(bt + 1) * P], x0_16[bt][:], 128)
            for m in range(3):
                transpose128(x1s[m][0:64, bt * P:(bt + 1) * P],
                             x1g16[bt][:, 64 * m:64 * (m + 1)], 64)
            for m in range(5):
                transpose128(x2s[m][0:32, bt * P:(bt + 1) * P],
                             x2g16[bt][:, 32 * m:32 * (m + 1)], 32)
        for m in range(3):
            nc.sync.dma_start(x1s[m][64:128, :], x1s[m][0:64, :])
        for m in range(5):
            for r in range(1, 4):
                nc.sync.dma_start(x2s[m][32 * r:32 * (r + 1), :], x2s[m][0:32, :])

        # ---------------- phase A: forward FCTP GEMM ----------------
        hpre16 = [sb.tile([P, HID], F16, name=f"hpre16_{bt}") for bt in range(NBT)]
        KT = 128 + 32 + 8
        with tc.tile_pool(name="fw_ps", bufs=1, space="PSUM") as fw_ps, \
             tc.tile_pool(name="bps", bufs=2, space="PSUM") as bps_pool, \
             tc.tile_pool(name="wpool", bufs=4) as wpool, \
             tc.tile_pool(name="b16p", bufs=4) as b16_pool, \
             tc.tile_pool(name="optp", bufs=8) as opt_pool:
            hpre_ps = [fw_ps.tile([P, HID], F32, name=f"hpre_ps{bt}")
                       for bt in range(NBT)]

            def emit_fwd_mms(opt_t, w_t, kk):
                for bt in range(NBT):
                    for hb in range(2):
                        nc.tensor.matmul(
                            hpre_ps[bt][:, hb * 512:(hb + 1) * 512],
                            opt_t[:, bt * P:(bt + 1) * P],
                            w_t[:, hb * 512:(hb + 1) * 512],
                            start=(kk == 0), stop=(kk == KT - 1),
                            skip_group_check=True)

            def build_opt_multi(xs, ubase, kdim, nm):
                """OPT tile = sum_m xs[m] * row-broadcasts of xs[m] rows
                ubase..ubase+nseg (nseg = 128//kdim), via PE matmuls."""
                nseg = P // kdim
                opt_t = opt_pool.tile([P, B], F16, name="opt_t", tag="opt")
                for m in range(len(xs)):
                    bps = bps_pool.tile([P, B], F32, name="bps", tag="bps")
                    for seg in range(nseg):
                        col = ubase + seg
                        nc.tensor.matmul(
                            bps[seg * kdim:(seg + 1) * kdim, :],
                            ident[0:kdim, col:col + 1].broadcast_to([kdim, kdim]),
                            xs[m][0:kdim, :],
                            tile_position=(0, seg * kdim),
                            skip_group_check=True)
                    b16 = b16_pool.tile([P, B], F16, name="b16", tag="b16")
                    nc.scalar.copy(b16[:], bps[:])
                    if m == 0:
                        nc.vector.tensor_mul(opt_t[:], xs[m][:], b16[:])
                    else:
                        tmp = b16_pool.tile([P, B], F16, name="tmp16", tag="b16")
                        nc.vector.tensor_mul(tmp[:], xs[m][:], b16[:])
                        nc.gpsimd.tensor_add(opt_t[:], opt_t[:], tmp[:])
                return opt_t

            kk = 0
            for u in range(128):
                w_t = wpool.tile([P, HID], F16, name="w_t", tag="w")
                nc.sync.dma_start(w_t[:], w0r[u * P:(u + 1) * P, :])
                opt_t = build_opt_multi([x0T], u, 128, "l0")
                emit_fwd_mms(opt_t, w_t, kk); kk += 1
            for j in range(32):
                w_t = wpool.tile([P, HID], F16, name="w_t", tag="w")
                nc.sync.dma_start(w_t[:], w1r[j * P:(j + 1) * P, :])
                opt_t = build_opt_multi(x1s, 2 * j, 64, "l1")
                emit_fwd_mms(opt_t, w_t, kk); kk += 1
            for j in range(8):
                w_t = wpool.tile([P, HID], F16, name="w_t", tag="w")
                nc.sync.dma_start(w_t[:], w2r[j * P:(j + 1) * P, :])
                opt_t = build_opt_multi(x2s, 4 * j, 32, "l2")
                emit_fwd_mms(opt_t, w_t, kk); kk += 1
            assert kk == KT

            for bt in range(NBT):
                nc.vector.tensor_copy(hpre16[bt][:], hpre_ps[bt][:])

        hpreT = [sb.tile([P, B], F16, name=f"hpreT{k}") for k in range(8)]
        for bt in range(NBT):
            for k in range(8):
                transpose128(hpreT[k][:, bt * P:(bt + 1) * P],
                             hpre16[bt][:, k * P:(k + 1) * P], 128)

        # resident small MLP weights
        W2sb = [sb.tile([P, Z], F16, name=f"W2sb{k}") for k in range(8)]
        W2Tsb = [sb.tile([P, HID], F16, name=f"W2Tsb{k}") for k in range(2)]
        for k in range(8):
            nc.sync.dma_start(W2sb[k][:], W2d[k * P:(k + 1) * P, :])
        for k in range(2):
            nc.sync.dma_start(W2Tsb[k][:], W2Td[k * P:(k + 1) * P, :])

        def layer_norm(a_ps, bias_bc, gain_bc, beta_bc, width, tag, xh_tile):
            """a = psum + bias; returns (xh into xh_tile, rs, l)."""
            a_t = scr.tile([P, width], F32, name=f"a_{tag}", tag="ln_a")
            sumx = tiny.tile([P, 1], F32, name=f"sumx_{tag}")
            nc.vector.scalar_tensor_tensor(
                out=a_t[:], in0=a_ps[:], scalar=1.0, in1=bias_bc[:],
                op0=OP.mult, op1=OP.add, accum_out=sumx[:])
            sq = scr.tile([P, width], F16, name=f"sq_{tag}", tag="ln_sq")
            sumsq = tiny.tile([P, 1], F32, name=f"sumsq_{tag}")
            nc.scalar.activation(sq[:], a_t[:], AF.Square, accum_out=sumsq[:])
            mu = tiny.tile([P, 1], F32, name=f"mu_{tag}")
            nc.vector.tensor_scalar_mul(mu[:], sumx[:], 1.0 / width)
            var = tiny.tile([P, 1], F32, name=f"var_{tag}")
            nc.vector.tensor_scalar_mul(var[:], sumsq[:], 1.0 / width)
            musq = tiny.tile([P, 1], F32, name=f"musq_{tag}")
            nc.vector.tensor_mul(musq[:], mu[:], mu[:])
            nc.vector.tensor_sub(var[:], var[:], musq[:])
            sd = tiny.tile([P, 1], F32, name=f"sd_{tag}")
            nc.scalar.activation(sd[:], var[:], AF.Sqrt, bias=eps_ap[:])
            rs = tiny.tile([P, 1], F32, name=f"rs_{tag}")
            nc.vector.reciprocal(rs[:], sd[:])
            nmrs = tiny.tile([P, 1], F32, name=f"nmrs_{tag}")
            nc.vector.tensor_mul(nmrs[:], mu[:], rs[:])
            nc.vector.tensor_scalar_mul(nmrs[:], nmrs[:], -1.0)
            nc.scalar.activation(xh_tile[:], a_t[:], AF.Identity,
                                 bias=nmrs[:], scale=rs[:])
            l_t = scr.tile([P, width], F32, name=f"l_{tag}", tag="ln_l")
            nc.vector.tensor_mul(l_t[:], xh_tile[:], gain_bc[:])
            nc.vector.tensor_add(l_t[:], l_t[:], beta_bc[:])
            return rs, l_t

        # ---------------- phase B: MLP fwd + LN/silu + bwd head ----------------
        xh1 = [sb.tile([P, HID], F32, name=f"xh1_{bt}") for bt in range(NBT)]
        sig1 = [sb.tile([P, HID], F32, name=f"sig1_{bt}") for bt in range(NBT)]
        s116 = [sb.tile([P, HID], F16, name=f"s116_{bt}") for bt in range(NBT)]
        s1T = [sb.tile([P, B], F16, name=f"s1T{k}") for k in range(8)]
        da2T = [sb.tile([P, B], F16, name=f"da2T{k}") for k in range(2)]
        da1T = [sb.tile([P, B], F16, name=f"da1T{k}") for k in range(8)]
        rs1 = [None] * NBT

        with tc.tile_pool(name="mlp_ps", bufs=2, space="PSUM") as mlp_ps, \
             tc.tile_pool(name="w1pool", bufs=3) as w1pool:
            # GEMM2: a1 = hpre @ W1 + b1 (stream W1, k-outer)
            a1_ps = [mlp_ps.tile([P, HID], F32, name=f"a1_ps{bt}", tag="big")
                     for bt in range(NBT)]
            for k in range(8):
                w1t = w1pool.tile([P, HID], F16, name="w1t", tag="w1")
                nc.sync.dma_start(w1t[:], W1d[k * P:(k + 1) * P, :])
                for bt in range(NBT):
                    for hb in range(2):
                        nc.tensor.matmul(
                            a1_ps[bt][:, hb * 512:(hb + 1) * 512],
                            hpreT[k][:, bt * P:(bt + 1) * P],
                            w1t[:, hb * 512:(hb + 1) * 512],
                            start=(k == 0), stop=(k == 7))
            for bt in range(NBT):
                rs, l_t = layer_norm(a1_ps[bt], b1bc, g1bc, be1bc, HID,
                                     f"l1b{bt}", xh1[bt])
                rs1[bt] = rs
                nc.scalar.activation(sig1[bt][:], l_t[:], AF.Sigmoid)
                nc.vector.tensor_mul(s116[bt][:], l_t[:], sig1[bt][:])
                for k in range(8):
                    transpose128(s1T[k][:, bt * P:(bt + 1) * P],
                                 s116[bt][:, k * P:(k + 1) * P], 128)

            # layer 2 fwd + LN2 + silu + head bwd
            for bt in range(NBT):
                a2_ps = mlp_ps.tile([P, Z], F32, name="a2_ps", tag="small")
                for k in range(8):
                    nc.tensor.matmul(a2_ps[:], s1T[k][:, bt * P:(bt + 1) * P],
                                     W2sb[k][:], start=(k == 0), stop=(k == 7))
                xh2 = scr.tile([P, Z], F32, name="xh2", tag="xh2")
                rs2, l2 = layer_norm(a2_ps, b2bc, g2bc, be2bc, Z, f"l2b{bt}", xh2)
                sig2 = scr.tile([P, Z], F32, name="sig2", tag="sig2")
                nc.scalar.activation(sig2[:], l2[:], AF.Sigmoid)
                xo = scr.tile([P, Z], F32, name="xo", tag="xo")
                nc.vector.tensor_mul(xo[:], l2[:], sig2[:])
                nc.sync.dma_start(xout_d[bt * P:(bt + 1) * P, :], xo[:])
                t1 = scr.tile([P, Z], F32, name="t1s", tag="t1s")
                nc.vector.tensor_mul(t1[:], xo[:], sig2[:])
                dl2 = scr.tile([P, Z], F32, name="dl2", tag="dl2")
                nc.vector.tensor_add(dl2[:], sig2[:], xo[:])
                nc.vector.tensor_sub(dl2[:], dl2[:], t1[:])
                p2 = scr.tile([P, Z], F32, name="p2", tag="p2")
                nc.vector.tensor_mul(p2[:], dl2[:], g2bc[:])
                m1s = tiny.tile([P, 1], F32, name=f"m1s2_{bt}")
                nc.vector.reduce_sum(m1s[:], p2[:], axis=AX.X)
                pxh = scr.tile([P, Z], F16, name="pxh2", tag="pxh2")
                nc.vector.tensor_mul(pxh[:], p2[:], xh2[:])
                m2s = tiny.tile([P, 1], F32, name=f"m2s2_{bt}")
                nc.vector.reduce_sum(m2s[:], pxh[:], axis=AX.X)
                m1 = tiny.tile([P, 1], F32, name=f"m1_2_{bt}")
                nc.vector.tensor_scalar_mul(m1[:], m1s[:], 1.0 / Z)
                m2 = tiny.tile([P, 1], F32, name=f"m2_2_{bt}")
                nc.vector.tensor_scalar_mul(m2[:], m2s[:], 1.0 / Z)
                q2 = scr.tile([P, Z], F32, name="q2", tag="dpre2")
                nc.vector.tensor_scalar_sub(q2[:], p2[:], m1[:])
                dpre = scr.tile([P, Z], F32, name="dpre2b", tag="p2b")
                nc.vector.scalar_tensor_tensor(
                    out=dpre[:], in0=xh2[:], scalar=m2[:], in1=q2[:],
                    op0=OP.mult, op1=OP.subtract)
                negrs2 = tiny.tile([P, 1], F32, name=f"negrs2_{bt}")
                nc.vector.tensor_scalar_mul(negrs2[:], rs2[:], -1.0)
                da216 = scr.tile([P, Z], F16, name="da216", tag="da216")
                nc.scalar.activation(da216[:], dpre[:], AF.Copy, scale=negrs2[:])
                for k in range(2):
                    transpose128(da2T[k][:, bt * P:(bt + 1) * P],
                                 da216[:, k * P:(k + 1) * P], 128)

            # backward through layer 1
            for bt in range(NBT):
                ds1_ps = mlp_ps.tile([P, HID], F32, name="ds1_ps", tag="big")
                for hb in range(2):
                    for k in range(2):
                        nc.tensor.matmul(
                            ds1_ps[:, hb * 512:(hb + 1) * 512],
                            da2T[k][:, bt * P:(bt + 1) * P],
                            W2Tsb[k][:, hb * 512:(hb + 1) * 512],
                            start=(k == 0), stop=(k == 1))
                t1 = scr.tile([P, HID], F32, name="t1b", tag="ln_a")
                nc.vector.tensor_mul(t1[:], s116[bt][:], sig1[bt][:])
                dsl = scr.tile([P, HID], F32, name="dsl", tag="ln_l")
                nc.vector.tensor_add(dsl[:], sig1[bt][:], s116[bt][:])
                nc.vector.tensor_sub(dsl[:], dsl[:], t1[:])
                dl1 = scr.tile([P, HID], F32, name="dl1", tag="dl1")
                nc.vector.tensor_mul(dl1[:], ds1_ps[:], dsl[:])
                p1 = scr.tile([P, HID], F32, name="p1", tag="p1")
                nc.vector.tensor_mul(p1[:], dl1[:], g1bc[:])
                m1s = tiny.tile([P, 1], F32, name=f"m1s1_{bt}")
                nc.vector.reduce_sum(m1s[:], p1[:], axis=AX.X)
                pxh = scr.tile([P, HID], F16, name="pxh1", tag="ln_sq")
                nc.vector.tensor_mul(pxh[:], p1[:], xh1[bt][:])
                m2s = tiny.tile([P, 1], F32, name=f"m2s1_{bt}")
                nc.vector.reduce_sum(m2s[:], pxh[:], axis=AX.X)
                m1 = tiny.tile([P, 1], F32, name=f"m1_1_{bt}")
                nc.vector.tensor_scalar_mul(m1[:], m1s[:], 1.0 / HID)
                m2 = tiny.tile([P, 1], F32, name=f"m2_1_{bt}")
                nc.vector.tensor_scalar_mul(m2[:], m2s[:], 1.0 / HID)
                q1 = scr.tile([P, HID], F32, name="q1", tag="dpre1")
                nc.vector.tensor_scalar_sub(q1[:], p1[:], m1[:])
                dpre = scr.tile([P, HID], F32, name="dpre1b", tag="p1b")
                nc.vector.scalar_tensor_tensor(
                    out=dpre[:], in0=xh1[bt][:], scalar=m2[:], in1=q1[:],
                    op0=OP.mult, op1=OP.subtract)
                negrs1 = tiny.tile([P, 1], F32, name=f"negrs1_{bt}")
                nc.vector.tensor_scalar_mul(negrs1[:], rs1[bt][:], -1.0)
                da116 = scr.tile([P, HID], F16, name="da116", tag="da116")
                nc.scalar.activation(da116[:], dpre[:], AF.Copy, scale=negrs1[:])
                for k in range(8):
                    transpose128(da1T[k][:, bt * P:(bt + 1) * P],
                                 da116[:, k * P:(k + 1) * P], 128)

        # ---------------- phase C: g_hT GEMM ----------------
        ghT = [sb.tile([P, B], F16, name=f"ghT{i}") for i in range(8)]
        with tc.tile_pool(name="gh_ps", bufs=4, space="PSUM") as gh_psp, \
             tc.tile_pool(name="w1tpool", bufs=3) as w1tpool:
            for grp in range(2):
                ps_list = [gh_psp.tile([P, B], F32, name=f"gh_ps{i}", tag="gh")
                           for i in range(4)]
                for k in range(8):
                    w1tt = w1tpool.tile([P, B * 2], F16, name="w1tt", tag="w1t")
                    nc.sync.dma_start(
                        w1tt[:], W1Td[k * P:(k + 1) * P, grp * 512:(grp + 1) * 512])
                    for ii in range(4):
                        nc.tensor.matmul(
                            ps_list[ii][:],
                            w1tt[:, ii * P:(ii + 1) * P],
                            da1T[k][:], start=(k == 0), stop=(k == 7))
                for ii in range(4):
                    nc.scalar.copy(ghT[grp * 4 + ii][:], ps_list[ii][:])

        # ---------------- phase D: S GEMMs + mul-reduce ----------------
        y_sb = [sb.tile([P, DIM], F32, name=f"y_sb{bt}") for bt in range(NBT)]
        with tc.tile_pool(name="s_ps", bufs=4, space="PSUM") as s_ps_pool, \
             tc.tile_pool(name="ws_pool", bufs=2) as ws_pool, \
             tc.tile_pool(name="red", bufs=4) as red_pool:

            def s_gemm_block(wsd, nb):
                wst = ws_pool.tile([P, 8, 512], F16, name="wst", tag="wst")
                nc.sync.dma_start(
                    wst[:],
                    wsd[:, nb * 512:(nb + 1) * 512].rearrange("(k p) c -> p k c", p=P))
                out = []
                for bt in range(NBT):
                    ps = s_ps_pool.tile([P, 512], F32, name="s_ps_t", tag="sps")
                    for k in range(8):
                        nc.tensor.matmul(ps[:], ghT[k][:, bt * P:(bt + 1) * P],
                                         wst[:, k, :], start=(k == 0), stop=(k == 7))
                    out.append(ps)
                return out

            for nb in range(32):   # l0: u in [4nb, 4nb+4), v 128
                s_ps = s_gemm_block(w0s, nb)
                for bt in range(NBT):
                    s16 = red_pool.tile([P, 512], F16, name="s16", tag="s16")
                    nc.scalar.copy(s16[:], s_ps[bt][:])
                    tmp = red_pool.tile([P, 4, 128], F16, name="tmpr", tag="tmpr")
                    nc.gpsimd.tensor_mul(
                        tmp[:], s16[:].rearrange("p (u v) -> p u v", u=4),
                        x0_16[bt][:].unsqueeze(1).broadcast_to([P, 4, 128]))
                    nc.vector.reduce_sum(y_sb[bt][:, 4 * nb:4 * nb + 4],
                                         tmp[:], axis=AX.X)
            y1v = [y_sb[bt][:, 128:320].rearrange("p (u m) -> p m u", m=3)
                   for bt in range(NBT)]
            for nb in range(8):    # l1: u in [8nb, 8nb+8), v 64
                s_ps = s_gemm_block(w1s, nb)
                for bt in range(NBT):
                    s16 = red_pool.tile([P, 512], F16, name="s16", tag="s16")
                    nc.scalar.copy(s16[:], s_ps[bt][:])
                    for m in range(3):
                        tmp = red_pool.tile([P, 8, 64], F16, name="tmpr1", tag="tmpr")
                        nc.gpsimd.tensor_mul(
                            tmp[:], s16[:].rearrange("p (u v) -> p u v", u=8),
                            x1g16[bt][:, 64 * m:64 * (m + 1)]
                            .unsqueeze(1).broadcast_to([P, 8, 64]))
                        nc.vector.reduce_sum(y1v[bt][:, m, 8 * nb:8 * (nb + 1)],
                                             tmp[:], axis=AX.X)
            y2v = [y_sb[bt][:, 320:480].rearrange("p (u m) -> p m u", m=5)
                   for bt in range(NBT)]
            for nb in range(2):    # l2: u in [16nb, 16nb+16), v 32
                s_ps = s_gemm_block(w2s, nb)
                for bt in range(NBT):
                    s16 = red_pool.tile([P, 512], F16, name="s16", tag="s16")
                    nc.scalar.copy(s16[:], s_ps[bt][:])
                    for m in range(5):
                        tmp = red_pool.tile([P, 16, 32], F16, name="tmpr2", tag="tmpr")
                        nc.gpsimd.tensor_mul(
                            tmp[:], s16[:].rearrange("p (u v) -> p u v", u=16),
                            x2g16[bt][:, 32 * m:32 * (m + 1)]
                            .unsqueeze(1).broadcast_to([P, 16, 32]))
                        nc.vector.reduce_sum(y2v[bt][:, m, 16 * nb:16 * (nb + 1)],
                                             tmp[:], axis=AX.X)

            for bt in range(NBT):
                nc.sync.dma_start(y_d[bt * P:(bt + 1) * P, :], y_sb[bt][:])


def _split_waits(nc):
    """The walrus build in this container encodes at most ONE sync-wait per
    instruction. Peel extra waits onto same-engine NoOps placed just before
    (engines execute in order, so this is semantically identical)."""
    import concourse.mybir as mybir
    import bass_rust

    n = 0
    for blk in nc.main_func.blocks:
        out = []
        for ins in blk.instructions:
            si = ins.sync_info
            waits = list(si.on_wait) if si and si.on_wait else []
            if len(waits) > 1:
                for w in waits[:-1]:
                    nop = mybir.InstNoOp(name=f"nopw-{n}", ins=[], outs=[])
                    n += 1
                    nop.engine = ins.engine
                    nop.sync_info = bass_rust.SyncInfo(on_wait=[w], on_update=[])
                    out.append(nop)
                ins.sync_info = bass_rust.SyncInfo(
                    on_wait=[waits[-1]], on_update=list(si.on_update))
            out.append(ins)
        blk.instructions[:] = out
    return n


def build_program(num_devices=NCORES):
    import concourse.bass as bass
    import concourse.tile as tile
    import concourse.mybir as mybir
    from concourse import masks

    nc = bass.Bass("TRN2", target_bir_lowering=False, debug=False,
                   num_devices=num_devices)
    with tile.TileContext(nc) as tc:
        _emit(nc, tc, tile, mybir, masks)
    _split_waits(nc)
    return nc


def prep_inputs(tensor_in, w0, w1, w2, W1, b1, g1, be1, W2, b2, g2, be2):
    sF = math.sqrt(float(FAN))
    f16 = np.float16
    w0_ = np.asarray(w0) / sF
    w1_ = np.asarray(w1) / (math.sqrt(3.0) * sF)
    w2_ = np.asarray(w2) / (math.sqrt(5.0) * sF)
    common = {
        "w0r": np.ascontiguousarray(w0_.reshape(16384, HID).astype(f16)),
        "w1r": np.ascontiguousarray(w1_.reshape(4096, HID).astype(f16)),
        "w2r": np.ascontiguousarray(w2_.reshape(1024, HID).astype(f16)),
        "w0s": np.ascontiguousarray(
            (w0_ + w0_.transpose(1, 0, 2)).reshape(16384, HID).T.astype(f16)),
        "w1s": np.ascontiguousarray(
            (w1_ + w1_.transpose(1, 0, 2)).reshape(4096, HID).T.astype(f16)),
        "w2s": np.ascontiguousarray(
            (w2_ + w2_.transpose(1, 0, 2)).reshape(1024, HID).T.astype(f16)),
        "W1d": np.ascontiguousarray(np.asarray(W1).astype(f16)),
        "W1Td": np.ascontiguousarray(np.asarray(W1).T.astype(f16)),
        "W2d": np.ascontiguousarray(np.asarray(W2).astype(f16)),
        "W2Td": np.ascontiguousarray(np.asarray(W2).T.astype(f16)),
        "b1d": np.ascontiguousarray(np.asarray(b1, np.float32).reshape(1, HID)),
        "g1d": np.ascontiguousarray(np.asarray(g1, np.float32).reshape(1, HID)),
        "be1d": np.ascontiguousarray(np.asarray(be1, np.float32).reshape(1, HID)),
        "b2d": np.ascontiguousarray(np.asarray(b2, np.float32).reshape(1, Z)),
        "g2d": np.ascontiguousarray(np.asarray(g2, np.float32).reshape(1, Z)),
        "be2d": np.ascontiguousarray(np.asarray(be2, np.float32).reshape(1, Z)),
        "identd": np.eye(128, dtype=np.float16),
    }
    t32 = np.ascontiguousarray(np.asarray(tensor_in, np.float32))
    in_maps = []
    for c in range(NCORES):
        m = dict(common)
        m["t_in"] = np.ascontiguousarray(t32[c * B:(c + 1) * B, :])
        in_maps.append(m)
    return in_maps


def kernel(**inputs):
    from concourse import bass_utils

    if "nc" not in _cache:
        _cache["nc"] = build_program()
    nc = _cache["nc"]
    in_maps = prep_inputs(**inputs)
    res = bass_utils.run_bass_kernel_spmd(nc, in_maps, core_ids=list(range(NCORES)))
    x = np.concatenate([res.results[c]["xout_d"] for c in range(NCORES)], axis=0)
    y = np.concatenate([res.results[c]["y_d"] for c in range(NCORES)], axis=0)
    return x.astype(np.float32), y.astype(np.float32)
